# revision 14
# baseline (speedup 1.0000x reference)
"""Trainium2 Bass kernel for AlphaFold-style gated attention.

Reference math (B=4, N=1024, C=512, H=8, CH=64):
    q = (q_x @ Wq) / 8 ; k = kv_x @ Wk ; v = kv_x @ Wv
    s = q k^T + bias_mask[b,k] + bias_pair[h,q,k]
    a = softmax_k(s) ; o = a @ v
    g = sigmoid(q_x @ Wg + bg)
    out = (o*g) @ Wo + bo

Sharding: 8 cores = (batch b in 0..3) x (q-half qh in 0..1). Zero collectives.

Device-side trick sheet:
  - All activations kept transposed ([feat on partitions, rows on free]);
    host pre-transposes inputs, so no on-chip transposes at all.
  - exp without max subtraction (scores are O(5), fp32 exp is safe).
  - bias_mask folded into v on host: kvem = kv_x * exp(mask)[k].
  - bias_pair folded as host-precomputed exp(pair)^T, multiplied into exp(s).
  - softmax denominator = extra em column in v -> free row in AV matmul.
  - 1/d broadcast across partitions via K=1 outer-product matmul into PSUM.
  - head PAIRS processed together: the two 64-contraction score matmuls use
    PE row groups 0-1/2-3 (concurrent on HW), share one [128,1024] PSUM
    tile, one Exp and one pair-multiply.
  - touch ops keep every instruction at <=1 foreign-semaphore wait.
"""

import sys

import numpy as np

if "/opt/trn_rl_repo" not in sys.path:
    sys.path.insert(0, "/opt/trn_rl_repo")

import ml_dtypes

import concourse.bass as bass
import concourse.tile as tile
from concourse import bacc, mybir
from concourse.bass_utils import run_bass_kernel_spmd

B, N, C, H, CH = 4, 1024, 512, 8, 64
R = 512          # q rows per core
KC = N // 128    # 8 k chunks of 128
CC = C // 128    # 4 feature chunks of 128
F32 = mybir.dt.float32
BF16 = mybir.dt.bfloat16
BF = ml_dtypes.bfloat16


def build(finalize=True):
    nc = bacc.Bacc("TRN2", target_bir_lowering=False, debug=False)

    qxt = nc.dram_tensor("qxt", [C, R], BF16, kind="ExternalInput").ap()
    kvt = nc.dram_tensor("kvt", [C, N], BF16, kind="ExternalInput").ap()
    kvem = nc.dram_tensor("kvem", [C, N], BF16, kind="ExternalInput").ap()
    emb = nc.dram_tensor("emb", [128, KC], BF16, kind="ExternalInput").ap()
    pairt = nc.dram_tensor("pairt", [H, N, R], BF16, kind="ExternalInput").ap()
    wq = nc.dram_tensor("wq", [C, C], BF16, kind="ExternalInput").ap()
    wk = nc.dram_tensor("wk", [C, C], BF16, kind="ExternalInput").ap()
    wv = nc.dram_tensor("wv", [C, C], BF16, kind="ExternalInput").ap()
    wg = nc.dram_tensor("wg", [C, C], BF16, kind="ExternalInput").ap()
    wo = nc.dram_tensor("wo", [C, C], BF16, kind="ExternalInput").ap()
    bgr = nc.dram_tensor("bgr", [128, CC], F32, kind="ExternalInput").ap()
    bor = nc.dram_tensor("bor", [128, CC], F32, kind="ExternalInput").ap()
    out = nc.dram_tensor("out", [C, R], BF16, kind="ExternalOutput").ap()

    with tile.TileContext(nc) as tc:
        _body(tc, qxt, kvt, kvem, emb, pairt, wq, wk, wv, wg, wo, bgr, bor, out)
    if finalize:
        nc.finalize()
    return nc


def _body(tc, qxt, kvt, kvem, emb, pairt, wq, wk, wv, wg, wo, bgr, bor, out):
    nc = tc.nc
    Exp = mybir.ActivationFunctionType.Exp
    Sigmoid = mybir.ActivationFunctionType.Sigmoid
    Ident = mybir.ActivationFunctionType.Identity

    with (
        tc.tile_pool(name="keep", bufs=1) as keep,
        tc.tile_pool(name="sb", bufs=3) as sb,
        tc.tile_pool(name="pairp", bufs=2) as pairp,
        tc.tile_pool(name="dp", bufs=3) as dp,
        tc.tile_pool(name="outp", bufs=2) as outp,
        tc.tile_pool(name="psA", bufs=2, space="PSUM") as psA,
        tc.tile_pool(name="psO", bufs=2, space="PSUM") as psO,
        tc.tile_pool(name="psD", bufs=1, space="PSUM") as psD,
    ):
        # ---- issue input DMAs (q-projection inputs first) ----
        w_sb = {}
        for wname in ("wq", "wk", "wv", "wg", "wo"):
            w_sb[wname] = keep.tile([128, CC, C], BF16, tag=wname,
                                    name=f"w_{wname}")
        qxt_sb = keep.tile([128, CC, R], BF16, tag="qxt")
        kvt_sb = keep.tile([128, CC, N], BF16, tag="kvt")
        kvem_sb = keep.tile([128, CC, N], BF16, tag="kvem")
        bgr_sb = keep.tile([128, CC], F32, tag="bgr")
        bor_sb = keep.tile([128, CC], F32, tag="bor")

        qxt_r = qxt.rearrange("(cc p) r -> cc p r", p=128)
        wq_r = wq.rearrange("(cc p) o -> cc p o", p=128)
        for ci in range(CC):
            nc.sync.dma_start(out=qxt_sb[:, ci, :], in_=qxt_r[ci])
            nc.sync.dma_start(out=w_sb["wq"][:, ci, :], in_=wq_r[ci])
        nc.sync.dma_start(out=kvt_sb, in_=kvt.rearrange("(cc p) n -> p cc n", p=128))
        nc.sync.dma_start(out=w_sb["wk"], in_=wk.rearrange("(cc p) o -> p cc o", p=128))
        nc.sync.dma_start(out=w_sb["wg"], in_=wg.rearrange("(cc p) o -> p cc o", p=128))
        nc.sync.dma_start(out=kvem_sb, in_=kvem.rearrange("(cc p) n -> p cc n", p=128))
        nc.sync.dma_start(out=w_sb["wv"], in_=wv.rearrange("(cc p) o -> p cc o", p=128))
        nc.sync.dma_start(out=bgr_sb, in_=bgr)
        nc.sync.dma_start(out=bor_sb, in_=bor)

        ones_64 = keep.tile([1, 64], BF16, tag="ones_64")
        nc.vector.memset(ones_64, 1.0)
        tps = psD.tile([1, 8], F32, tag="touch")
        scr = keep.tile([1, 2], F32, tag="scr")
        scr2 = keep.tile([1, 1], BF16, tag="scr2")

        qT = keep.tile([128, CC, R], BF16, tag="qT")
        kT = keep.tile([128, CC, N], BF16, tag="kT")
        gT = keep.tile([128, CC, R], F32, tag="gT")
        vS = keep.tile([128, KC, H, 65], BF16, tag="vS")

        # ---- q projection (per-chunk DMAs: each matmul waits <=2 lanes) ----
        for cc in range(CC):
            ps = psA.tile([128, R], F32, tag="ps")
            for ci in range(CC):
                nc.tensor.matmul(
                    ps, w_sb["wq"][:, ci, cc * 128:(cc + 1) * 128],
                    qxt_sb[:, ci, :], start=(ci == 0), stop=(ci == CC - 1))
            nc.vector.tensor_copy(qT[:, cc, :], ps)

        # ---- k projection ----
        nc.tensor.matmul(tps[0:1, 1:2], w_sb["wk"][0:1, 0, 0:1],
                         kvt_sb[0:1, 0, 0:1], start=True, stop=True)
        for cc in range(CC):
            for nh in range(2):
                ps2 = psA.tile([128, R], F32, tag="ps")
                for ci in range(CC):
                    nc.tensor.matmul(
                        ps2, w_sb["wk"][:, ci, cc * 128:(cc + 1) * 128],
                        kvt_sb[:, ci, nh * 512:(nh + 1) * 512],
                        start=(ci == 0), stop=(ci == CC - 1))
                nc.vector.tensor_copy(kT[:, cc, nh * 512:(nh + 1) * 512], ps2)

        # ---- g projection (sigmoid with bias) ----
        nc.tensor.matmul(tps[0:1, 2:3], w_sb["wg"][0:1, 0, 0:1],
                         qxt_sb[0:1, 0, 0:1], start=True, stop=True)
        nc.scalar.activation(scr[0:1, 0:1], bgr_sb[0:1, 0:1], Ident)
        nc.scalar.activation(scr[0:1, 1:2], bor_sb[0:1, 0:1], Ident)
        for cc in range(CC):
            ps3 = psA.tile([128, R], F32, tag="ps")
            for ci in range(CC):
                nc.tensor.matmul(
                    ps3, w_sb["wg"][:, ci, cc * 128:(cc + 1) * 128],
                    qxt_sb[:, ci, :], start=(ci == 0), stop=(ci == CC - 1))
            nc.scalar.activation(gT[:, cc, :], ps3, Sigmoid,
                                 bias=bgr_sb[:, cc:cc + 1])

        # ---- pairt double-buffer prefetch (pair 0 queued before v-proj) ----
        pair_tiles = {}

        def fetch_pair(hp):
            h0 = 2 * hp
            t = pairp.tile([128, KC, 2, R], BF16, tag="pair",
                           name=f"pairt_{hp}")
            for hi in range(2):
                nc.sync.dma_start(
                    out=t[:, :, hi, :],
                    in_=pairt[h0 + hi].rearrange("(kc p) r -> p kc r", p=128))
            pair_tiles[hp] = t

        fetch_pair(0)
        nc.sync.dma_start(out=w_sb["wo"], in_=wo.rearrange("(cc p) o -> p cc o", p=128))

        # ---- v (natural layout from em-scaled kv_x), em column via DMA ----
        nc.tensor.matmul(tps[0:1, 3:4], w_sb["wv"][0:1, 0, 0:1],
                         kvem_sb[0:1, 0, 0:1], start=True, stop=True)
        for kc in range(KC):
            ps = psA.tile([128, R], F32, tag="ps")
            for ci in range(CC):
                nc.tensor.matmul(
                    ps, kvem_sb[:, ci, kc * 128:(kc + 1) * 128],
                    w_sb["wv"][:, ci, :], start=(ci == 0), stop=(ci == CC - 1))
            nc.vector.tensor_copy(vS[:, kc, :, 0:64], ps)
        emb_bcast = bass.AP(tensor=emb.tensor, offset=emb.offset,
                            ap=[[KC, 128], [1, KC], [0, H]])
        nc.gpsimd.dma_start(out=vS[:, :, :, 64:65], in_=emb_bcast)

        # ---- attention, head pairs (row groups 0-1 / 2-3 on PE) ----
        # Software-pipelined: scores for kc issue one step ahead of the AV
        # matmuls (PE is in-order; the lookahead hides the exp+mult chain),
        # and each pair's normalize/gate epilogue is deferred past the next
        # pair's first scores so the db matmuls never stall the PE.
        xgT = keep.tile([128, CC, R], BF16, tag="xgT")
        pending = None

        def emit_epilogue(hp, ov0, ov1, dinvs):
            for hi, ov, dinv in ((0, ov0, dinvs[0]), (1, ov1, dinvs[1])):
                po = hi * 64
                db = psD.tile([64, R], F32, tag="db", name=f"db_{hp}_{hi}")
                nc.tensor.matmul(db, ones_64, dinv, start=True, stop=True)
                gd = dp.tile([64, R], F32, tag="gd", name=f"gd_{hp}_{hi}")
                nc.vector.tensor_mul(gd, gT[po:po + 64, hp, :], db)
                nc.vector.tensor_mul(xgT[po:po + 64, hp, :], ov[0:64, :], gd)

        for hp in range(H // 2):
            h0 = 2 * hp
            pairt_h = pair_tiles[hp]
            nc.vector.tensor_copy(scr2, pairt_h[0:1, 0, 0, 0:1])
            nc.vector.tensor_copy(scr2, pairt_h[0:1, 0, 1, 0:1])
            if hp + 1 < H // 2:
                fetch_pair(hp + 1)

            ov0 = psO.tile([65, R], F32, tag="ov")
            ov1 = psO.tile([65, R], F32, tag="ov")
            a_prev = None
            for kc in range(KC):
                st = psA.tile([128, 2, R], F32, tag="ps")
                nc.tensor.matmul(
                    st[:, 0, :], kT[0:64, hp, kc * 128:(kc + 1) * 128],
                    qT[0:64, hp, :], start=True, stop=True)
                nc.tensor.matmul(
                    st[:, 1, :], kT[64:128, hp, kc * 128:(kc + 1) * 128],
                    qT[64:128, hp, :], start=True, stop=True)
                if kc == 1 and pending is not None:
                    emit_epilogue(*pending)
                    pending = None
                e = sb.tile([128, 2, R], BF16, tag="e")
                nc.scalar.activation(e, st, Exp)
                a_t = sb.tile([128, 2, R], BF16, tag="at")
                nc.vector.tensor_mul(a_t, e, pairt_h[:, kc, :, :])
                if a_prev is not None:
                    pk = kc - 1
                    nc.tensor.matmul(ov0, vS[:, pk, h0, :], a_prev[:, 0, :],
                                     start=(pk == 0), stop=False)
                    nc.tensor.matmul(ov1, vS[:, pk, h0 + 1, :],
                                     a_prev[:, 1, :],
                                     start=(pk == 0), stop=False)
                a_prev = a_t
            nc.tensor.matmul(ov0, vS[:, KC - 1, h0, :], a_prev[:, 0, :],
                             start=False, stop=True)
            nc.tensor.matmul(ov1, vS[:, KC - 1, h0 + 1, :], a_prev[:, 1, :],
                             start=False, stop=True)

            dinvs = []
            for hi, ov in ((0, ov0), (1, ov1)):
                dinv = dp.tile([1, R], BF16, tag="dinv",
                               name=f"dinv_{hp}_{hi}")
                with nc.allow_low_precision(reason="1/d bf16 for bcast mm"):
                    nc.vector.reciprocal(dinv, ov[64:65, :])
                dinvs.append(dinv)
            pending = (hp, ov0, ov1, dinvs)
        emit_epilogue(*pending)

        # ---- output projection + bias ----
        nc.tensor.matmul(tps[0:1, 4:5], w_sb["wo"][0:1, 0, 0:1],
                         w_sb["wo"][0:1, 0, 0:1], start=True, stop=True)
        out_r = out.rearrange("(cc p) r -> cc p r", p=128)
        for cc in range(CC):
            ps = psA.tile([128, R], F32, tag="ps")
            for ci in range(CC):
                nc.tensor.matmul(
                    ps, w_sb["wo"][:, ci, cc * 128:(cc + 1) * 128],
                    xgT[:, ci, :], start=(ci == 0), stop=(ci == CC - 1))
            osb = outp.tile([128, R], BF16, tag="out")
            nc.scalar.activation(osb, ps, Ident, bias=bor_sb[:, cc:cc + 1])
            nc.sync.dma_start(out=out_r[cc], in_=osb)


def prep_in_maps(q_x, kv_x, bias_mask, bias_pair, Wq, Wk, Wv, Wg, bg, Wo, bo):
    f32 = np.float32
    shared = {
        "wq": np.ascontiguousarray((np.asarray(Wq, f32) * 0.125).astype(BF)),
        "wk": np.ascontiguousarray(np.asarray(Wk, f32).astype(BF)),
        "wv": np.ascontiguousarray(np.asarray(Wv, f32).astype(BF)),
        "wg": np.ascontiguousarray(np.asarray(Wg, f32).astype(BF)),
        "wo": np.ascontiguousarray(np.asarray(Wo, f32).astype(BF)),
        "bgr": np.ascontiguousarray(
            np.asarray(bg, f32).reshape(CC, 128).T),
        "bor": np.ascontiguousarray(
            np.asarray(bo, f32).reshape(CC, 128).T),
    }
    pair_exp_t = {}
    bp = np.asarray(bias_pair, f32)[0]  # [H, N, N] (h, q, k)
    for qh in (0, 1):
        sl = bp[:, qh * R:(qh + 1) * R, :]          # [H, R(q), N(k)]
        pair_exp_t[qh] = np.ascontiguousarray(
            np.exp(sl).transpose(0, 2, 1).astype(BF))  # [H, N(k), R(q)]

    in_maps = []
    for i in range(8):
        b, qh = i // 2, i % 2
        m = dict(shared)
        m["qxt"] = np.ascontiguousarray(
            np.asarray(q_x[b, qh * R:(qh + 1) * R, :], f32).T.astype(BF))
        m["kvt"] = np.ascontiguousarray(np.asarray(kv_x[b], f32).T.astype(BF))
        em = np.exp(np.asarray(bias_mask[b, 0, 0], f32))
        m["kvem"] = np.ascontiguousarray(
            (np.asarray(kv_x[b], f32) * em[:, None]).T.astype(BF))
        m["emb"] = np.ascontiguousarray(em.reshape(KC, 128).T.astype(BF))
        m["pairt"] = pair_exp_t[qh]
        in_maps.append(m)
    return in_maps


def assemble(results):
    out = np.empty((B, N, C), np.float32)
    for i, r in enumerate(results):
        b, qh = i // 2, i % 2
        out[b, qh * R:(qh + 1) * R, :] = np.asarray(r["out"], np.float32).T
    return out


def kernel(q_x, kv_x, bias_mask, bias_pair, Wq, Wk, Wv, Wg, bg, Wo, bo):
    nc = build()
    in_maps = prep_in_maps(q_x, kv_x, bias_mask, bias_pair,
                           Wq, Wk, Wv, Wg, bg, Wo, bo)
    res = run_bass_kernel_spmd(nc, in_maps, core_ids=list(range(8)))
    return assemble(res.results)


if __name__ == "__main__":
    nc = build()
    print("build OK")


# revision 15
# speedup vs baseline: 23470.2832x; 23470.2832x over previous
"""Trainium2 Bass kernel for AlphaFold-style gated attention.

Reference math (B=4, N=1024, C=512, H=8, CH=64):
    q = (q_x @ Wq) / 8 ; k = kv_x @ Wk ; v = kv_x @ Wv
    s = q k^T + bias_mask[b,k] + bias_pair[h,q,k]
    a = softmax_k(s) ; o = a @ v
    g = sigmoid(q_x @ Wg + bg)
    out = (o*g) @ Wo + bo

Sharding: 8 cores = (batch b in 0..3) x (q-half qh in 0..1). Zero collectives.

Device-side trick sheet:
  - All activations kept transposed ([feat on partitions, rows on free]);
    host pre-transposes inputs, so no on-chip transposes at all.
  - exp without max subtraction (scores are O(5), fp32 exp is safe).
  - bias_mask folded into v on host: kvem = kv_x * exp(mask)[k].
  - bias_pair folded as host-precomputed exp(pair)^T, multiplied into exp(s).
  - softmax denominator = extra em column in v -> free row in AV matmul.
  - 1/d broadcast across partitions via K=1 outer-product matmul into PSUM.
  - head PAIRS processed together: the two 64-contraction score matmuls use
    PE row groups 0-1/2-3 (concurrent on HW), share one [128,1024] PSUM
    tile, one Exp and one pair-multiply.
  - touch ops keep every instruction at <=1 foreign-semaphore wait.
"""

import sys

import numpy as np

if "/opt/trn_rl_repo" not in sys.path:
    sys.path.insert(0, "/opt/trn_rl_repo")

import ml_dtypes

import concourse.bass as bass
import concourse.tile as tile
from concourse import bacc, mybir
from concourse.bass_utils import run_bass_kernel_spmd

B, N, C, H, CH = 4, 1024, 512, 8, 64
R = 512          # q rows per core
KC = N // 128    # 8 k chunks of 128
CC = C // 128    # 4 feature chunks of 128
F32 = mybir.dt.float32
BF16 = mybir.dt.bfloat16
BF = ml_dtypes.bfloat16


def build(finalize=True):
    nc = bacc.Bacc("TRN2", target_bir_lowering=False, debug=False)

    qxt = nc.dram_tensor("qxt", [C, R], BF16, kind="ExternalInput").ap()
    kvt = nc.dram_tensor("kvt", [C, N], BF16, kind="ExternalInput").ap()
    kvem = nc.dram_tensor("kvem", [C, N], BF16, kind="ExternalInput").ap()
    emb = nc.dram_tensor("emb", [128, KC], BF16, kind="ExternalInput").ap()
    pairt = nc.dram_tensor("pairt", [H, N, R], BF16, kind="ExternalInput").ap()
    wq = nc.dram_tensor("wq", [C, C], BF16, kind="ExternalInput").ap()
    wk = nc.dram_tensor("wk", [C, C], BF16, kind="ExternalInput").ap()
    wv = nc.dram_tensor("wv", [C, C], BF16, kind="ExternalInput").ap()
    wg = nc.dram_tensor("wg", [C, C], BF16, kind="ExternalInput").ap()
    wo = nc.dram_tensor("wo", [C, C], BF16, kind="ExternalInput").ap()
    bgr = nc.dram_tensor("bgr", [128, CC], F32, kind="ExternalInput").ap()
    bor = nc.dram_tensor("bor", [128, CC], F32, kind="ExternalInput").ap()
    out = nc.dram_tensor("out", [C, R], BF16, kind="ExternalOutput").ap()

    with tile.TileContext(nc) as tc:
        _body(tc, qxt, kvt, kvem, emb, pairt, wq, wk, wv, wg, wo, bgr, bor, out)
    if finalize:
        nc.finalize()
    return nc


def _body(tc, qxt, kvt, kvem, emb, pairt, wq, wk, wv, wg, wo, bgr, bor, out):
    nc = tc.nc
    Exp = mybir.ActivationFunctionType.Exp
    Sigmoid = mybir.ActivationFunctionType.Sigmoid
    Ident = mybir.ActivationFunctionType.Identity

    with (
        tc.tile_pool(name="keep", bufs=1) as keep,
        tc.tile_pool(name="sb", bufs=3) as sb,
        tc.tile_pool(name="pairp", bufs=2) as pairp,
        tc.tile_pool(name="dp", bufs=3) as dp,
        tc.tile_pool(name="outp", bufs=2) as outp,
        tc.tile_pool(name="psA", bufs=2, space="PSUM") as psA,
        tc.tile_pool(name="psO", bufs=2, space="PSUM") as psO,
        tc.tile_pool(name="psD", bufs=1, space="PSUM") as psD,
    ):
        # ---- issue input DMAs (q-projection inputs first) ----
        w_sb = {}
        for wname in ("wq", "wk", "wv", "wg", "wo"):
            w_sb[wname] = keep.tile([128, CC, C], BF16, tag=wname,
                                    name=f"w_{wname}")
        qxt_sb = keep.tile([128, CC, R], BF16, tag="qxt")
        kvt_sb = keep.tile([128, CC, N], BF16, tag="kvt")
        kvem_sb = keep.tile([128, CC, N], BF16, tag="kvem")
        bgr_sb = keep.tile([128, CC], F32, tag="bgr")
        bor_sb = keep.tile([128, CC], F32, tag="bor")

        qxt_r = qxt.rearrange("(cc p) r -> cc p r", p=128)
        wq_r = wq.rearrange("(cc p) o -> cc p o", p=128)
        for ci in range(CC):
            nc.sync.dma_start(out=qxt_sb[:, ci, :], in_=qxt_r[ci])
            nc.sync.dma_start(out=w_sb["wq"][:, ci, :], in_=wq_r[ci])
        nc.sync.dma_start(out=kvt_sb, in_=kvt.rearrange("(cc p) n -> p cc n", p=128))
        nc.sync.dma_start(out=w_sb["wk"], in_=wk.rearrange("(cc p) o -> p cc o", p=128))
        nc.sync.dma_start(out=w_sb["wg"], in_=wg.rearrange("(cc p) o -> p cc o", p=128))
        nc.sync.dma_start(out=kvem_sb, in_=kvem.rearrange("(cc p) n -> p cc n", p=128))
        nc.sync.dma_start(out=w_sb["wv"], in_=wv.rearrange("(cc p) o -> p cc o", p=128))
        nc.sync.dma_start(out=bgr_sb, in_=bgr)
        nc.sync.dma_start(out=bor_sb, in_=bor)

        ones_64 = keep.tile([1, 64], BF16, tag="ones_64")
        nc.vector.memset(ones_64, 1.0)
        tps = psD.tile([1, 8], F32, tag="touch")
        scr = keep.tile([1, 2], F32, tag="scr")
        scr2 = keep.tile([1, 1], BF16, tag="scr2")

        qT = keep.tile([128, CC, R], BF16, tag="qT")
        kT = keep.tile([128, CC, N], BF16, tag="kT")
        gT = keep.tile([128, CC, R], F32, tag="gT")
        vS = keep.tile([128, KC, H, 65], BF16, tag="vS")

        # ---- q projection (per-chunk DMAs: each matmul waits <=2 lanes) ----
        for cc in range(CC):
            ps = psA.tile([128, R], F32, tag="ps")
            for ci in range(CC):
                nc.tensor.matmul(
                    ps, w_sb["wq"][:, ci, cc * 128:(cc + 1) * 128],
                    qxt_sb[:, ci, :], start=(ci == 0), stop=(ci == CC - 1))
            nc.vector.tensor_copy(qT[:, cc, :], ps)

        # ---- k projection ----
        nc.tensor.matmul(tps[0:1, 1:2], w_sb["wk"][0:1, 0, 0:1],
                         kvt_sb[0:1, 0, 0:1], start=True, stop=True)
        for cc in range(CC):
            for nh in range(2):
                ps2 = psA.tile([128, R], F32, tag="ps")
                for ci in range(CC):
                    nc.tensor.matmul(
                        ps2, w_sb["wk"][:, ci, cc * 128:(cc + 1) * 128],
                        kvt_sb[:, ci, nh * 512:(nh + 1) * 512],
                        start=(ci == 0), stop=(ci == CC - 1))
                nc.vector.tensor_copy(kT[:, cc, nh * 512:(nh + 1) * 512], ps2)

        # ---- g projection (sigmoid with bias) ----
        nc.tensor.matmul(tps[0:1, 2:3], w_sb["wg"][0:1, 0, 0:1],
                         qxt_sb[0:1, 0, 0:1], start=True, stop=True)
        nc.scalar.activation(scr[0:1, 0:1], bgr_sb[0:1, 0:1], Ident)
        nc.scalar.activation(scr[0:1, 1:2], bor_sb[0:1, 0:1], Ident)
        for cc in range(CC):
            ps3 = psA.tile([128, R], F32, tag="ps")
            for ci in range(CC):
                nc.tensor.matmul(
                    ps3, w_sb["wg"][:, ci, cc * 128:(cc + 1) * 128],
                    qxt_sb[:, ci, :], start=(ci == 0), stop=(ci == CC - 1))
            nc.scalar.activation(gT[:, cc, :], ps3, Sigmoid,
                                 bias=bgr_sb[:, cc:cc + 1])

        # ---- pairt double-buffer prefetch (pair 0 queued before v-proj) ----
        pair_tiles = {}

        def fetch_pair(hp):
            h0 = 2 * hp
            t = pairp.tile([128, KC, 2, R], BF16, tag="pair",
                           name=f"pairt_{hp}")
            for hi in range(2):
                nc.sync.dma_start(
                    out=t[:, :, hi, :],
                    in_=pairt[h0 + hi].rearrange("(kc p) r -> p kc r", p=128))
            pair_tiles[hp] = t

        fetch_pair(0)
        nc.sync.dma_start(out=w_sb["wo"], in_=wo.rearrange("(cc p) o -> p cc o", p=128))

        # ---- v (natural layout from em-scaled kv_x), em column via DMA ----
        nc.tensor.matmul(tps[0:1, 3:4], w_sb["wv"][0:1, 0, 0:1],
                         kvem_sb[0:1, 0, 0:1], start=True, stop=True)
        for kc in range(KC):
            ps = psA.tile([128, R], F32, tag="ps")
            for ci in range(CC):
                nc.tensor.matmul(
                    ps, kvem_sb[:, ci, kc * 128:(kc + 1) * 128],
                    w_sb["wv"][:, ci, :], start=(ci == 0), stop=(ci == CC - 1))
            nc.vector.tensor_copy(vS[:, kc, :, 0:64], ps)
        emb_bcast = bass.AP(tensor=emb.tensor, offset=emb.offset,
                            ap=[[KC, 128], [1, KC], [0, H]])
        nc.gpsimd.dma_start(out=vS[:, :, :, 64:65], in_=emb_bcast)

        # ---- attention, head pairs (row groups 0-1 / 2-3 on PE) ----
        # Software-pipelined: scores for kc issue one step ahead of the AV
        # matmuls (PE is in-order; the lookahead hides the exp+mult chain),
        # and each pair's normalize/gate epilogue is deferred past the next
        # pair's first scores so the db matmuls never stall the PE.
        xgT = keep.tile([128, CC, R], BF16, tag="xgT")
        pending = None

        def emit_epilogue(hp, ov0, ov1, dinvs):
            for hi, ov, dinv in ((0, ov0, dinvs[0]), (1, ov1, dinvs[1])):
                po = hi * 64
                db = psD.tile([64, R], F32, tag="db", name=f"db_{hp}_{hi}")
                nc.tensor.matmul(db, ones_64, dinv, start=True, stop=True)
                gd = dp.tile([64, R], F32, tag="gd", name=f"gd_{hp}_{hi}")
                nc.vector.tensor_mul(gd, gT[po:po + 64, hp, :], db)
                nc.vector.tensor_mul(xgT[po:po + 64, hp, :], ov[0:64, :], gd)

        for hp in range(H // 2):
            h0 = 2 * hp
            pairt_h = pair_tiles[hp]
            nc.vector.tensor_copy(scr2, pairt_h[0:1, 0, 0, 0:1])
            nc.vector.tensor_copy(scr2, pairt_h[0:1, 0, 1, 0:1])
            if hp + 1 < H // 2:
                fetch_pair(hp + 1)

            ov0 = psO.tile([65, R], F32, tag="ov")
            ov1 = psO.tile([65, R], F32, tag="ov")
            a_prev = None
            for kc in range(KC):
                st = psA.tile([128, 2, R], F32, tag="ps")
                nc.tensor.matmul(
                    st[:, 0, :], kT[0:64, hp, kc * 128:(kc + 1) * 128],
                    qT[0:64, hp, :], start=True, stop=True)
                nc.tensor.matmul(
                    st[:, 1, :], kT[64:128, hp, kc * 128:(kc + 1) * 128],
                    qT[64:128, hp, :], start=True, stop=True)
                if kc == 1 and pending is not None:
                    emit_epilogue(*pending)
                    pending = None
                e = sb.tile([128, 2, R], BF16, tag="e")
                nc.scalar.activation(e, st, Exp)
                a_t = sb.tile([128, 2, R], BF16, tag="at")
                nc.vector.tensor_mul(a_t, e, pairt_h[:, kc, :, :])
                if a_prev is not None:
                    pk = kc - 1
                    nc.tensor.matmul(ov0, vS[:, pk, h0, :], a_prev[:, 0, :],
                                     start=(pk == 0), stop=False)
                    nc.tensor.matmul(ov1, vS[:, pk, h0 + 1, :],
                                     a_prev[:, 1, :],
                                     start=(pk == 0), stop=False)
                a_prev = a_t
            nc.tensor.matmul(ov0, vS[:, KC - 1, h0, :], a_prev[:, 0, :],
                             start=False, stop=True)
            nc.tensor.matmul(ov1, vS[:, KC - 1, h0 + 1, :], a_prev[:, 1, :],
                             start=False, stop=True)

            dinvs = []
            for hi, ov in ((0, ov0), (1, ov1)):
                dinv = dp.tile([1, R], BF16, tag="dinv",
                               name=f"dinv_{hp}_{hi}")
                with nc.allow_low_precision(reason="1/d bf16 for bcast mm"):
                    nc.vector.reciprocal(dinv, ov[64:65, :])
                dinvs.append(dinv)
                if hp == H // 2 - 1:
                    # last pair: emit this head's epilogue immediately so the
                    # chain overlaps the other head's reciprocal
                    po = hi * 64
                    db = psD.tile([64, R], F32, tag="db",
                                  name=f"db_l_{hi}")
                    nc.tensor.matmul(db, ones_64, dinv, start=True, stop=True)
                    gd = dp.tile([64, R], F32, tag="gd", name=f"gd_l_{hi}")
                    nc.vector.tensor_mul(gd, gT[po:po + 64, hp, :], db)
                    nc.vector.tensor_mul(xgT[po:po + 64, hp, :],
                                         ov[0:64, :], gd)
            if hp < H // 2 - 1:
                pending = (hp, ov0, ov1, dinvs)

        # ---- output projection + bias ----
        nc.tensor.matmul(tps[0:1, 4:5], w_sb["wo"][0:1, 0, 0:1],
                         w_sb["wo"][0:1, 0, 0:1], start=True, stop=True)
        out_r = out.rearrange("(cc p) r -> cc p r", p=128)
        for cc in range(CC):
            ps = psA.tile([128, R], F32, tag="ps")
            for ci in range(CC):
                nc.tensor.matmul(
                    ps, w_sb["wo"][:, ci, cc * 128:(cc + 1) * 128],
                    xgT[:, ci, :], start=(ci == 0), stop=(ci == CC - 1))
            osb = outp.tile([128, R], BF16, tag="out")
            nc.scalar.activation(osb, ps, Ident, bias=bor_sb[:, cc:cc + 1])
            nc.sync.dma_start(out=out_r[cc], in_=osb)


def prep_in_maps(q_x, kv_x, bias_mask, bias_pair, Wq, Wk, Wv, Wg, bg, Wo, bo):
    f32 = np.float32
    shared = {
        "wq": np.ascontiguousarray((np.asarray(Wq, f32) * 0.125).astype(BF)),
        "wk": np.ascontiguousarray(np.asarray(Wk, f32).astype(BF)),
        "wv": np.ascontiguousarray(np.asarray(Wv, f32).astype(BF)),
        "wg": np.ascontiguousarray(np.asarray(Wg, f32).astype(BF)),
        "wo": np.ascontiguousarray(np.asarray(Wo, f32).astype(BF)),
        "bgr": np.ascontiguousarray(
            np.asarray(bg, f32).reshape(CC, 128).T),
        "bor": np.ascontiguousarray(
            np.asarray(bo, f32).reshape(CC, 128).T),
    }
    pair_exp_t = {}
    bp = np.asarray(bias_pair, f32)[0]  # [H, N, N] (h, q, k)
    for qh in (0, 1):
        sl = bp[:, qh * R:(qh + 1) * R, :]          # [H, R(q), N(k)]
        pair_exp_t[qh] = np.ascontiguousarray(
            np.exp(sl).transpose(0, 2, 1).astype(BF))  # [H, N(k), R(q)]

    in_maps = []
    for i in range(8):
        b, qh = i // 2, i % 2
        m = dict(shared)
        m["qxt"] = np.ascontiguousarray(
            np.asarray(q_x[b, qh * R:(qh + 1) * R, :], f32).T.astype(BF))
        m["kvt"] = np.ascontiguousarray(np.asarray(kv_x[b], f32).T.astype(BF))
        em = np.exp(np.asarray(bias_mask[b, 0, 0], f32))
        m["kvem"] = np.ascontiguousarray(
            (np.asarray(kv_x[b], f32) * em[:, None]).T.astype(BF))
        m["emb"] = np.ascontiguousarray(em.reshape(KC, 128).T.astype(BF))
        m["pairt"] = pair_exp_t[qh]
        in_maps.append(m)
    return in_maps


def assemble(results):
    out = np.empty((B, N, C), np.float32)
    for i, r in enumerate(results):
        b, qh = i // 2, i % 2
        out[b, qh * R:(qh + 1) * R, :] = np.asarray(r["out"], np.float32).T
    return out


def kernel(q_x, kv_x, bias_mask, bias_pair, Wq, Wk, Wv, Wg, bg, Wo, bo):
    nc = build()
    in_maps = prep_in_maps(q_x, kv_x, bias_mask, bias_pair,
                           Wq, Wk, Wv, Wg, bg, Wo, bo)
    res = run_bass_kernel_spmd(nc, in_maps, core_ids=list(range(8)))
    return assemble(res.results)


if __name__ == "__main__":
    nc = build()
    print("build OK")
